# revision 14
# baseline (speedup 1.0000x reference)
"""Dense multi-head attention (B=4, H=16, N=2048, D=64) on 8 trn2 NeuronCores.

Sharding: batch*head parallel - 64 (b,h) pairs, 8 per core. Each core runs a
fused flash-style attention over its heads.

v2 kernel design (PE moving-port roofline):
  The PE moving-read port sustains ~256B / 0.417ns; a matmul's per-column
  cost is max(moving bytes, psum-write bytes/2)/port. Exploit:
  - S^T matmuls emit 64-partition outputs (256B/col psum, 128B/col moving):
    two 64-wide k-blocks per PSUM tile via tile_position (0,0)/(0,64),
    streaming 2 cols/cycle -> ~226ns per [128k x 512q] pair (vs 427ns).
  - O matmuls [65, 512] accumulate (PSUM RMW is free) at ~234ns.
  - exp: ScalarE exact exp on cols [0,SPLIT); DVE Schraudolph bit-trick exp
    (int16 bits of bf16 = rint(a*s + b)) on cols [SPLIT,1024) - writes via an
    int16-aliased view of the bf16 probs tile.
  - epilogue off-PE: DVE f32->bf16 copy + reciprocal row, XBAR DMA transpose
    [64,1024]->[128,8,64], denominator row scattered to [128,8], Pool
    tensor_scalar per-partition normalize. No PE transposes, no identity.
No max-subtraction pass: scores/8 ~ N(0,1); exp stays well inside f32/bf16
range, matching jax.nn.softmax to bf16 precision.
"""

import os
import sys

import numpy as np

for _p in ("/opt/trn_rl_repo", "/root/.axon_site/_ro/trn_rl_repo"):
    if os.path.isdir(_p) and _p not in sys.path:
        sys.path.insert(0, _p)

import ml_dtypes

B, H, N, D = 4, 16, 2048, 64
NCORES = 8
HPC = B * H // NCORES  # heads (b,h pairs) per core = 8
BF16 = ml_dtypes.bfloat16

# Schraudolph exp: bf16 bits ~= rint(A_SCH * s + B_SCH) for exp(s/8)
A_SCH = 0.125 * float(np.log2(np.e)) * 128.0  # 23.083120
B_SCH = 16256.0 - 7.4  # 127*128 + fitted log-centering correction
SPLIT = 1024  # cols [0,SPLIT) exact exp on ScalarE; rest Schraudolph on DVE

_CACHE = {}


def _build_nc(split=SPLIT):
    import concourse.bass as bass
    import concourse.mybir as mybir
    import concourse.tile as tile
    from concourse import bacc

    bf16 = mybir.dt.bfloat16
    f32 = mybir.dt.float32
    i16 = mybir.dt.int16

    QC = 1024         # q chunk (PSUM: [128, QC] f32 = 2 banks)
    NQC = N // QC     # 2 q-chunks per head
    MP = N // 128     # 16 k-pair blocks (2 x 64) per head
    QB = QC // 128    # 8 128-row q blocks per chunk

    nc = bacc.Bacc(
        "TRN2", target_bir_lowering=False, debug=False, num_devices=NCORES
    )
    qt = nc.declare_dram_parameter("qt", [HPC, D, N], bf16, isOutput=False)
    kt = nc.declare_dram_parameter("kt", [HPC, D, N], bf16, isOutput=False)
    va = nc.declare_dram_parameter("va", [HPC, N, D + 1], bf16, isOutput=False)
    out = nc.declare_dram_parameter("out", [HPC, N, D], bf16, isOutput=True)

    with tile.TileContext(nc) as tc:
        with (
            tc.sbuf_pool(name="inp", bufs=2) as inp,
            tc.sbuf_pool(name="probs", bufs=5) as probs,
            tc.sbuf_pool(name="epil", bufs=2) as epil,
            tc.psum_pool(name="spsum", bufs=2) as spsum,
            tc.psum_pool(name="opsum", bufs=2) as opsum,
        ):
            def emit_head(h):
                qt_t = inp.tile([D, N], bf16, tag="qt", name="qt_t")
                nc.sync.dma_start(out=qt_t, in_=qt[h])
                kt_t = inp.tile([D, N], bf16, tag="kt", name="kt_t")
                nc.sync.dma_start(out=kt_t, in_=kt[h])
                va_t = inp.tile([128, MP, D + 1], bf16, tag="va", name="va_t")
                nc.sync.dma_start(
                    out=va_t, in_=va[h].rearrange("(m p) d -> p m d", p=128)
                )
                out_t = epil.tile([128, N // 128, D], bf16, tag="out", name="out_t")

                for qc in range(NQC):
                    o_ps = opsum.tile([D + 1, QC], f32, tag="o", name="o_ps")
                    pend = []

                    def emit_o(mp, p_t):
                        for u in range(QC // 512):
                            nc.tensor.matmul(
                                o_ps[:, u * 512 : (u + 1) * 512],
                                va_t[:, mp, :],
                                p_t[:, u * 512 : (u + 1) * 512],
                                start=(mp == 0),
                                stop=(mp == MP - 1),
                            )

                    for mp in range(MP):
                        s_ps = spsum.tile([128, QC], f32, tag="s", name="s_ps")
                        # 2 S matmuls: [64,128] stationary -> [128,512] out
                        # (uniform full-array tile config, same as O mms)
                        st = kt_t[:, mp * 128 : (mp + 1) * 128]
                        for u in range(QC // 512):
                            nc.tensor.matmul(
                                s_ps[:, u * 512 : (u + 1) * 512],
                                st,
                                qt_t[:, qc * QC + u * 512 : qc * QC + (u + 1) * 512],
                                start=True,
                                stop=True,
                            )
                        p_t = probs.tile([128, QC], bf16, tag="p", name="p_t")
                        if split > 0:
                            nc.scalar.activation(
                                p_t[:, 0:split],
                                s_ps[:, 0:split],
                                mybir.ActivationFunctionType.Exp,
                                scale=0.125,
                            )
                        if split < QC:
                            # int16-aliased view of p_t for Schraudolph bits
                            ht = p_t.tensor
                            h16 = bass.SBTensorHandle(
                                ht.name, list(ht.shape), i16, base_partition=0
                            )
                            p16 = h16.ap()[:, split:QC]
                            nc.vector.tensor_scalar(
                                p16,
                                s_ps[:, split:QC],
                                A_SCH,
                                B_SCH,
                                mybir.AluOpType.mult,
                                mybir.AluOpType.add,
                            )
                        pend.append((mp, p_t))
                        if len(pend) > 3:
                            omp, op = pend.pop(0)
                            emit_o(omp, op)
                    for omp, op in pend:
                        emit_o(omp, op)
                    # epilogue (off-PE)
                    obf = epil.tile([D + 1, QC], bf16, tag="obf", name="obf")
                    nc.vector.tensor_copy(obf, o_ps)
                    rec = epil.tile([1, QC], f32, tag="rec", name="rec")
                    nc.vector.reciprocal(rec, o_ps[D : D + 1, :])
                    o_T = epil.tile([128, QB, D], bf16, tag="oT", name="o_T")
                    nc.scalar.dma_start_transpose(o_T, obf[0:D, :])
                    recT = epil.tile([128, QB], f32, tag="recT", name="recT")
                    for j in range(QB):
                        nc.sync.dma_start(
                            out=recT[:, j : j + 1],
                            in_=rec[:, j * 128 : (j + 1) * 128],
                        )
                    for j in range(QB):
                        nc.gpsimd.tensor_scalar_mul(
                            out_t[:, qc * QB + j, :],
                            o_T[:, j, :],
                            recT[:, j : j + 1],
                        )
                nc.sync.dma_start(
                    out=out[h].rearrange("(m p) d -> p m d", p=128), in_=out_t
                )

            for h in range(HPC):
                emit_head(h)
    nc.compile()
    return nc


def _get_nc():
    if "nc" not in _CACHE:
        _CACHE["nc"] = _build_nc()
    return _CACHE["nc"]


def _prep_shards(q, k, v):
    """Host-side: split heads, cast bf16 (round-to-nearest-even, matching the
    reference's astype), transpose Q/K to [d, n], append ones column to V."""
    q4 = np.ascontiguousarray(
        q.reshape(B, N, H, D).transpose(0, 2, 3, 1).reshape(B * H, D, N)
    ).astype(BF16)
    k4 = np.ascontiguousarray(
        k.reshape(B, N, H, D).transpose(0, 2, 3, 1).reshape(B * H, D, N)
    ).astype(BF16)
    v4 = np.ascontiguousarray(
        v.reshape(B, N, H, D).transpose(0, 2, 1, 3).reshape(B * H, N, D)
    ).astype(BF16)
    ones = np.ones((B * H, N, 1), dtype=BF16)
    va = np.concatenate([v4, ones], axis=2)

    in_maps = []
    for c in range(NCORES):
        sl = slice(c * HPC, (c + 1) * HPC)
        in_maps.append(
            {
                "qt": np.ascontiguousarray(q4[sl]),
                "kt": np.ascontiguousarray(k4[sl]),
                "va": np.ascontiguousarray(va[sl]),
            }
        )
    return in_maps


def _make_runner():
    """Persistent jitted SPMD executor (mirrors bass2jax.run_bass_via_pjrt but
    reusable across calls, no donation so device inputs can be reused)."""
    import jax
    import numpy as _np
    from jax.sharding import Mesh, PartitionSpec
    from concourse import bass2jax, mybir

    try:
        from jax.experimental.shard_map import shard_map
    except ImportError:
        shard_map = jax.shard_map

    bass2jax.install_neuronx_cc_hook()
    nc = _get_nc()

    partition_name = (
        nc.partition_id_tensor.name if nc.partition_id_tensor is not None else None
    )
    in_names, out_names, out_avals, zero_outs = [], [], [], []
    for alloc in nc.m.functions[0].allocations:
        if not isinstance(alloc, mybir.MemoryLocationSet):
            continue
        name = alloc.memorylocations[0].name
        if alloc.kind == "ExternalInput":
            if name != partition_name:
                in_names.append(name)
        elif alloc.kind == "ExternalOutput":
            out_names.append(name)
            shape = tuple(alloc.tensor_shape)
            dtype = mybir.dt.np(alloc.dtype)
            out_avals.append(jax.core.ShapedArray(shape, dtype))
            zero_outs.append(_np.zeros(shape, dtype))
    n_params = len(in_names)

    all_in_names = in_names + out_names
    if partition_name is not None:
        all_in_names = all_in_names + [partition_name]

    def _body(*args):
        operands = list(args)
        if partition_name is not None:
            operands.append(bass2jax.partition_id_tensor())
        outs = bass2jax._bass_exec_p.bind(
            *operands,
            out_avals=tuple(out_avals),
            in_names=tuple(all_in_names),
            out_names=tuple(out_names),
            lowering_input_output_aliases=(),
            sim_require_finite=True,
            sim_require_nnan=True,
            nc=nc,
        )
        return tuple(outs)

    devices = jax.devices()[:NCORES]
    mesh = Mesh(np.asarray(devices), ("core",))
    in_specs = (PartitionSpec("core"),) * (n_params + len(out_names))
    out_specs = (PartitionSpec("core"),) * len(out_names)
    sharded = jax.jit(
        shard_map(
            _body, mesh=mesh, in_specs=in_specs, out_specs=out_specs, check_rep=False
        ),
        keep_unused=True,
    )

    def run(in_maps):
        concat_in = [
            np.concatenate([in_maps[c][nm] for c in range(NCORES)], axis=0)
            for nm in in_names
        ]
        concat_zeros = [
            np.zeros((NCORES * z.shape[0], *z.shape[1:]), z.dtype) for z in zero_outs
        ]
        out_arrs = sharded(*concat_in, *concat_zeros)
        return [
            {
                nm: np.asarray(out_arrs[i]).reshape(NCORES, *out_avals[i].shape)[c]
                for i, nm in enumerate(out_names)
            }
            for c in range(NCORES)
        ]

    def put(in_maps):
        import jax as _jax
        from jax.sharding import NamedSharding

        sh = NamedSharding(mesh, PartitionSpec("core"))
        concat_in = [
            np.concatenate([in_maps[c][nm] for c in range(NCORES)], axis=0)
            for nm in in_names
        ]
        concat_zeros = [
            np.zeros((NCORES * z.shape[0], *z.shape[1:]), z.dtype) for z in zero_outs
        ]
        return [_jax.device_put(x, sh) for x in concat_in + concat_zeros]

    return {"run": run, "put": put, "sharded": sharded}


def _get_runner():
    if "runner" not in _CACHE:
        _CACHE["runner"] = _make_runner()
    return _CACHE["runner"]


def timed_run(in_maps, iters=10):
    """Return (best_wall_seconds_per_call, results). Device-resident inputs."""
    import time

    import jax

    r = _get_runner()
    args = r["put"](in_maps)
    out = r["sharded"](*args)
    jax.block_until_ready(out)
    best = float("inf")
    for _ in range(iters):
        t0 = time.perf_counter()
        out = r["sharded"](*args)
        jax.block_until_ready(out)
        best = min(best, time.perf_counter() - t0)
    return best, out


def kernel(q, k, v):
    q = np.asarray(q, dtype=np.float32)
    k = np.asarray(k, dtype=np.float32)
    v = np.asarray(v, dtype=np.float32)
    in_maps = _prep_shards(q, k, v)

    res = _get_runner()["run"](in_maps)

    outs = [np.asarray(res[c]["out"]) for c in range(NCORES)]
    out_all = np.concatenate(outs, axis=0)  # [B*H, N, D] bf16
    full = (
        out_all.reshape(B, H, N, D).transpose(0, 2, 1, 3).reshape(B, N, H * D)
    )
    return np.ascontiguousarray(full)


# revision 15
# speedup vs baseline: 1.0086x; 1.0086x over previous
"""Dense multi-head attention (B=4, H=16, N=2048, D=64) on 8 trn2 NeuronCores.

Sharding: batch*head parallel - 64 (b,h) pairs, 8 per core. Each core runs a
fused flash-style attention over its heads.

v2 kernel design (PE moving-port roofline):
  The PE moving-read port sustains ~256B / 0.417ns; a matmul's per-column
  cost is max(moving bytes, psum-write bytes/2)/port. Exploit:
  - S^T matmuls emit 64-partition outputs (256B/col psum, 128B/col moving):
    two 64-wide k-blocks per PSUM tile via tile_position (0,0)/(0,64),
    streaming 2 cols/cycle -> ~226ns per [128k x 512q] pair (vs 427ns).
  - O matmuls [65, 512] accumulate (PSUM RMW is free) at ~234ns.
  - exp: ScalarE exact exp on cols [0,SPLIT); DVE Schraudolph bit-trick exp
    (int16 bits of bf16 = rint(a*s + b)) on cols [SPLIT,1024) - writes via an
    int16-aliased view of the bf16 probs tile.
  - epilogue off-PE: DVE f32->bf16 copy + reciprocal row, XBAR DMA transpose
    [64,1024]->[128,8,64], denominator row scattered to [128,8], Pool
    tensor_scalar per-partition normalize. No PE transposes, no identity.
No max-subtraction pass: scores/8 ~ N(0,1); exp stays well inside f32/bf16
range, matching jax.nn.softmax to bf16 precision.
"""

import os
import sys

import numpy as np

for _p in ("/opt/trn_rl_repo", "/root/.axon_site/_ro/trn_rl_repo"):
    if os.path.isdir(_p) and _p not in sys.path:
        sys.path.insert(0, _p)

import ml_dtypes

B, H, N, D = 4, 16, 2048, 64
NCORES = 8
HPC = B * H // NCORES  # heads (b,h pairs) per core = 8
BF16 = ml_dtypes.bfloat16

# Schraudolph exp: bf16 bits ~= rint(A_SCH * s + B_SCH) for exp(s/8)
A_SCH = 0.125 * float(np.log2(np.e)) * 128.0  # 23.083120
B_SCH = 16256.0 - 7.4  # 127*128 + fitted log-centering correction
SPLIT = 1024  # cols [0,SPLIT) exact exp on ScalarE; rest Schraudolph on DVE

_CACHE = {}


def _build_nc(split=SPLIT):
    import concourse.bass as bass
    import concourse.mybir as mybir
    import concourse.tile as tile
    from concourse import bacc

    bf16 = mybir.dt.bfloat16
    f32 = mybir.dt.float32
    i16 = mybir.dt.int16

    QC = 1024         # q chunk (PSUM: [128, QC] f32 = 2 banks)
    NQC = N // QC     # 2 q-chunks per head
    MP = N // 128     # 16 k-pair blocks (2 x 64) per head
    QB = QC // 128    # 8 128-row q blocks per chunk

    nc = bacc.Bacc(
        "TRN2", target_bir_lowering=False, debug=False, num_devices=NCORES
    )
    qt = nc.declare_dram_parameter("qt", [HPC, D, N], bf16, isOutput=False)
    kt = nc.declare_dram_parameter("kt", [HPC, D, N], bf16, isOutput=False)
    va = nc.declare_dram_parameter("va", [HPC, N, D + 1], bf16, isOutput=False)
    out = nc.declare_dram_parameter("out", [HPC, N, D], bf16, isOutput=True)

    with tile.TileContext(nc) as tc:
        with (
            tc.sbuf_pool(name="inp", bufs=2) as inp,
            tc.sbuf_pool(name="probs", bufs=4) as probs,
            tc.sbuf_pool(name="epil", bufs=2) as epil,
            tc.psum_pool(name="spsum", bufs=2) as spsum,
            tc.psum_pool(name="opsum", bufs=2) as opsum,
        ):
            def emit_head(h):
                qt_t = inp.tile([D, N], bf16, tag="qt", name="qt_t")
                nc.sync.dma_start(out=qt_t, in_=qt[h])
                kt_t = inp.tile([D, N], bf16, tag="kt", name="kt_t")
                nc.sync.dma_start(out=kt_t, in_=kt[h])
                va_t = inp.tile([128, MP, D + 1], bf16, tag="va", name="va_t")
                nc.sync.dma_start(
                    out=va_t, in_=va[h].rearrange("(m p) d -> p m d", p=128)
                )
                out_t = epil.tile([128, N // 128, D], bf16, tag="out", name="out_t")

                for qc in range(NQC):
                    o_ps = opsum.tile([D + 1, QC], f32, tag="o", name="o_ps")
                    pend = []

                    def emit_o(mp, p_t):
                        for u in range(QC // 512):
                            nc.tensor.matmul(
                                o_ps[:, u * 512 : (u + 1) * 512],
                                va_t[:, mp, :],
                                p_t[:, u * 512 : (u + 1) * 512],
                                start=(mp == 0),
                                stop=(mp == MP - 1),
                            )

                    for mp in range(MP):
                        s_ps = spsum.tile([128, QC], f32, tag="s", name="s_ps")
                        # 2 S matmuls: [64,128] stationary -> [128,512] out
                        # (uniform full-array tile config, same as O mms)
                        st = kt_t[:, mp * 128 : (mp + 1) * 128]
                        for u in range(QC // 512):
                            nc.tensor.matmul(
                                s_ps[:, u * 512 : (u + 1) * 512],
                                st,
                                qt_t[:, qc * QC + u * 512 : qc * QC + (u + 1) * 512],
                                start=True,
                                stop=True,
                            )
                        p_t = probs.tile([128, QC], bf16, tag="p", name="p_t")
                        if split > 0:
                            nc.scalar.activation(
                                p_t[:, 0:split],
                                s_ps[:, 0:split],
                                mybir.ActivationFunctionType.Exp,
                                scale=0.125,
                            )
                        if split < QC:
                            # int16-aliased view of p_t for Schraudolph bits
                            ht = p_t.tensor
                            h16 = bass.SBTensorHandle(
                                ht.name, list(ht.shape), i16, base_partition=0
                            )
                            p16 = h16.ap()[:, split:QC]
                            nc.vector.tensor_scalar(
                                p16,
                                s_ps[:, split:QC],
                                A_SCH,
                                B_SCH,
                                mybir.AluOpType.mult,
                                mybir.AluOpType.add,
                            )
                        pend.append((mp, p_t))
                        if len(pend) > 2:
                            omp, op = pend.pop(0)
                            emit_o(omp, op)
                    for omp, op in pend:
                        emit_o(omp, op)
                    # epilogue (off-PE)
                    obf = epil.tile([D + 1, QC], bf16, tag="obf", name="obf")
                    nc.vector.tensor_copy(obf, o_ps)
                    rec = epil.tile([1, QC], f32, tag="rec", name="rec")
                    nc.vector.reciprocal(rec, o_ps[D : D + 1, :])
                    o_T = epil.tile([128, QB, D], bf16, tag="oT", name="o_T")
                    nc.scalar.dma_start_transpose(o_T, obf[0:D, :])
                    recT = epil.tile([128, QB], f32, tag="recT", name="recT")
                    for j in range(QB):
                        nc.sync.dma_start(
                            out=recT[:, j : j + 1],
                            in_=rec[:, j * 128 : (j + 1) * 128],
                        )
                    for j in range(QB):
                        nc.gpsimd.tensor_scalar_mul(
                            out_t[:, qc * QB + j, :],
                            o_T[:, j, :],
                            recT[:, j : j + 1],
                        )
                nc.sync.dma_start(
                    out=out[h].rearrange("(m p) d -> p m d", p=128), in_=out_t
                )

            for h in range(HPC):
                emit_head(h)
    nc.compile()
    return nc


def _get_nc():
    if "nc" not in _CACHE:
        _CACHE["nc"] = _build_nc()
    return _CACHE["nc"]


def _prep_shards(q, k, v):
    """Host-side: split heads, cast bf16 (round-to-nearest-even, matching the
    reference's astype), transpose Q/K to [d, n], append ones column to V."""
    q4 = np.ascontiguousarray(
        q.reshape(B, N, H, D).transpose(0, 2, 3, 1).reshape(B * H, D, N)
    ).astype(BF16)
    k4 = np.ascontiguousarray(
        k.reshape(B, N, H, D).transpose(0, 2, 3, 1).reshape(B * H, D, N)
    ).astype(BF16)
    v4 = np.ascontiguousarray(
        v.reshape(B, N, H, D).transpose(0, 2, 1, 3).reshape(B * H, N, D)
    ).astype(BF16)
    ones = np.ones((B * H, N, 1), dtype=BF16)
    va = np.concatenate([v4, ones], axis=2)

    in_maps = []
    for c in range(NCORES):
        sl = slice(c * HPC, (c + 1) * HPC)
        in_maps.append(
            {
                "qt": np.ascontiguousarray(q4[sl]),
                "kt": np.ascontiguousarray(k4[sl]),
                "va": np.ascontiguousarray(va[sl]),
            }
        )
    return in_maps


def _make_runner():
    """Persistent jitted SPMD executor (mirrors bass2jax.run_bass_via_pjrt but
    reusable across calls, no donation so device inputs can be reused)."""
    import jax
    import numpy as _np
    from jax.sharding import Mesh, PartitionSpec
    from concourse import bass2jax, mybir

    try:
        from jax.experimental.shard_map import shard_map
    except ImportError:
        shard_map = jax.shard_map

    bass2jax.install_neuronx_cc_hook()
    nc = _get_nc()

    partition_name = (
        nc.partition_id_tensor.name if nc.partition_id_tensor is not None else None
    )
    in_names, out_names, out_avals, zero_outs = [], [], [], []
    for alloc in nc.m.functions[0].allocations:
        if not isinstance(alloc, mybir.MemoryLocationSet):
            continue
        name = alloc.memorylocations[0].name
        if alloc.kind == "ExternalInput":
            if name != partition_name:
                in_names.append(name)
        elif alloc.kind == "ExternalOutput":
            out_names.append(name)
            shape = tuple(alloc.tensor_shape)
            dtype = mybir.dt.np(alloc.dtype)
            out_avals.append(jax.core.ShapedArray(shape, dtype))
            zero_outs.append(_np.zeros(shape, dtype))
    n_params = len(in_names)

    all_in_names = in_names + out_names
    if partition_name is not None:
        all_in_names = all_in_names + [partition_name]

    def _body(*args):
        operands = list(args)
        if partition_name is not None:
            operands.append(bass2jax.partition_id_tensor())
        outs = bass2jax._bass_exec_p.bind(
            *operands,
            out_avals=tuple(out_avals),
            in_names=tuple(all_in_names),
            out_names=tuple(out_names),
            lowering_input_output_aliases=(),
            sim_require_finite=True,
            sim_require_nnan=True,
            nc=nc,
        )
        return tuple(outs)

    devices = jax.devices()[:NCORES]
    mesh = Mesh(np.asarray(devices), ("core",))
    in_specs = (PartitionSpec("core"),) * (n_params + len(out_names))
    out_specs = (PartitionSpec("core"),) * len(out_names)
    sharded = jax.jit(
        shard_map(
            _body, mesh=mesh, in_specs=in_specs, out_specs=out_specs, check_rep=False
        ),
        keep_unused=True,
    )

    def run(in_maps):
        concat_in = [
            np.concatenate([in_maps[c][nm] for c in range(NCORES)], axis=0)
            for nm in in_names
        ]
        concat_zeros = [
            np.zeros((NCORES * z.shape[0], *z.shape[1:]), z.dtype) for z in zero_outs
        ]
        out_arrs = sharded(*concat_in, *concat_zeros)
        return [
            {
                nm: np.asarray(out_arrs[i]).reshape(NCORES, *out_avals[i].shape)[c]
                for i, nm in enumerate(out_names)
            }
            for c in range(NCORES)
        ]

    def put(in_maps):
        import jax as _jax
        from jax.sharding import NamedSharding

        sh = NamedSharding(mesh, PartitionSpec("core"))
        concat_in = [
            np.concatenate([in_maps[c][nm] for c in range(NCORES)], axis=0)
            for nm in in_names
        ]
        concat_zeros = [
            np.zeros((NCORES * z.shape[0], *z.shape[1:]), z.dtype) for z in zero_outs
        ]
        return [_jax.device_put(x, sh) for x in concat_in + concat_zeros]

    return {"run": run, "put": put, "sharded": sharded}


def _get_runner():
    if "runner" not in _CACHE:
        _CACHE["runner"] = _make_runner()
    return _CACHE["runner"]


def timed_run(in_maps, iters=10):
    """Return (best_wall_seconds_per_call, results). Device-resident inputs."""
    import time

    import jax

    r = _get_runner()
    args = r["put"](in_maps)
    out = r["sharded"](*args)
    jax.block_until_ready(out)
    best = float("inf")
    for _ in range(iters):
        t0 = time.perf_counter()
        out = r["sharded"](*args)
        jax.block_until_ready(out)
        best = min(best, time.perf_counter() - t0)
    return best, out


def kernel(q, k, v):
    q = np.asarray(q, dtype=np.float32)
    k = np.asarray(k, dtype=np.float32)
    v = np.asarray(v, dtype=np.float32)
    in_maps = _prep_shards(q, k, v)

    res = _get_runner()["run"](in_maps)

    outs = [np.asarray(res[c]["out"]) for c in range(NCORES)]
    out_all = np.concatenate(outs, axis=0)  # [B*H, N, D] bf16
    full = (
        out_all.reshape(B, H, N, D).transpose(0, 2, 1, 3).reshape(B, N, H * D)
    )
    return np.ascontiguousarray(full)
